# revision 57
# baseline (speedup 1.0000x reference)
"""DialogueGCN forward as a Bass/Tile kernel on 8 TRN2 NeuronCores.

Sharding: data-parallel over dialogues (batch). Each core owns 32 contiguous
dialogues; edges never cross dialogues so all graph aggregation is local.

Key structure (per dialogue; u = source utterance, t = target, band |u-t|<=10):
  P[u,t]   = exp((W_att^T x^T)[u,t])       computed only on the band
  Shat_dd  = P * dir_dd                     (banded, bf16)
  sums[u]  = sum_t P*win  (via stt accums); xr_s = x * (msk_s/sums)   (bf16)
  G_{s,dd}[d,t] = sum_u xr_s[u,d] Shat_dd[u,t]   banded matmuls (bf16)
  h1_tau   = sum_{s,dd,ch} w8^T G  (+root^T x^T + bias), tau-select by
             target speaker via PE-broadcast mask + copy_predicated
  h2       = W1^T h1 + (h1^T W2)^T-banded-win + b_gc
  hid      = relu(Wlin^T [x;h2] + b);  logits = Wfc^T hid + b
  out      = log_softmax(logits) over 6 classes (batched stage 2)

All inputs are staged in SBUF by ~35 large DMAs (no per-dialogue DMA);
host pre-lays-out all tensors (incl. bf16 casts); output is one raw
(128, 576) DMA that the host reorders.
"""

import numpy as np
import ml_dtypes

import concourse.bass as bass
import concourse.mybir as mybir
import concourse.tile as tile
from concourse import bass_utils

SEQ, BATCH, D, H, NCLS = 300, 256, 200, 128, 6
WP = WF = 10
NCORES = 8
BPC = BATCH // NCORES  # dialogues per core
UT = [(0, 128), (128, 128), (256, 44)]   # u tiles (offset, size)
BND = [(0, 138), (118, 266), (246, 300)]  # per-u-tile t band [L, R)
BW = 148  # padded band width for constant mask tiles
F32 = mybir.dt.float32
F32R = mybir.dt.float32r
FP8 = mybir.dt.float8e4
BF16 = mybir.dt.bfloat16
NPBF16 = ml_dtypes.bfloat16
NPFP8 = ml_dtypes.float8_e4m3
SCL = 4096.0  # w8 x256, G x16; unfolded in the lg activation

# column-split plan for banded accumulation into a 300-col psum:
# (k, c0, c1, start, stop) with c0/c1 global t coords
GSPLIT = [
    (0, 0, 118, True, True),
    (0, 118, 138, True, False),
    (1, 118, 138, False, True),
    (1, 138, 246, True, True),
    (1, 246, 266, True, False),
    (2, 246, 266, False, True),
    (2, 266, 300, True, True),
]

_CACHE = {}


def _split_multiwaits(nc, max_waits=1):
    """walrus in this container rejects >1 sem wait on an instruction
    ("Too many sync wait commands"); hoist extras onto preceding NOPs."""
    n = 0
    for f in nc.m.functions:
        for b in f.blocks:
            newlist = []
            changed = False
            for ins in b.instructions:
                si = ins.sync_info
                if si is not None and si.on_wait is not None and len(si.on_wait) > max_waits:
                    waits = list(si.on_wait)
                    for w in waits[max_waits:]:
                        n += 1
                        nop = mybir.InstNoOp(name=f"waitsplit-{n}", ins=[], outs=[])
                        nop.engine = ins.engine
                        nop.sync_info = mybir.SyncInfo(on_wait=[w], on_update=[])
                        newlist.append(nop)
                        nc.inst_map[nop.name] = nop
                    ins.sync_info = mybir.SyncInfo(
                        on_wait=waits[:max_waits],
                        on_update=list(si.on_update) if si.on_update else [],
                    )
                    changed = True
                newlist.append(ins)
            if changed:
                b.instructions = newlist
    return n


def _build_program():
    nc = bass.Bass("TRN2", num_devices=NCORES)

    ap = {}
    def din(name, shape, dt=BF16):
        ap[name] = nc.dram_tensor(name, shape, dt, kind="ExternalInput").ap()

    din("xtb", (2, 101, BPC * SEQ))          # x^T d-chunk-major, all dialogues
    din("xnb", (3, 128, BPC * D))            # x u-tile-major (k2 zero-padded)
    din("mskb", (3, 128, 2 * BPC), F32)      # speaker one-hot per u-tile
    din("mskrow", (2, BPC * SEQ))            # speaker masks rows (bf16)
    din("dirb", (2, 3, 128, BW), F32)        # banded direction masks
    din("winb", (3, 128, BW))                # banded window mask (bf16)
    din("watt", (2, 100, 384))
    din("w8", (8, 100, 2 * H), FP8)
    din("rootm", (2, 101, H))
    din("w1m", (H, H))
    din("w2m", (H, H))
    din("wlind", (2, 101, H))
    din("wlinh", (H, H))
    din("wfc", (H, NCLS))
    din("bgc", (H, 1), F32)
    din("blc", (H, 1), F32)
    din("bfc8", (128, 8 * 3 * NCLS), F32)
    out = nc.dram_tensor("out", (128, BPC * 3 * NCLS), F32, kind="ExternalOutput").ap()

    from contextlib import ExitStack
    with tile.TileContext(nc) as tc:
        with ExitStack() as ctx:
            pools = dict(
                cpool=ctx.enter_context(tc.tile_pool(name="const", bufs=1)),
                wk=ctx.enter_context(tc.tile_pool(name="wk", bufs=3)),
                xpool=ctx.enter_context(tc.tile_pool(name="xpool", bufs=3)),
                fpool=ctx.enter_context(tc.tile_pool(name="fpool", bufs=3)),
                spool=ctx.enter_context(tc.tile_pool(name="spool", bufs=3)),
                gpool=ctx.enter_context(tc.tile_pool(name="gpool", bufs=5)),
                ps_sc=ctx.enter_context(tc.tile_pool(name="ps_sc", bufs=1, space="PSUM")),
                ps_g=ctx.enter_context(tc.tile_pool(name="ps_g", bufs=3, space="PSUM")),
                ps_h1=ctx.enter_context(tc.tile_pool(name="ps_h1", bufs=2, space="PSUM")),
                ps_ms=ctx.enter_context(tc.tile_pool(name="ps_ms", bufs=1, space="PSUM")),
            )
            _body(nc, tc, ap, out, pools)

    _split_multiwaits(nc)
    return nc


def _body(nc, tc, ap, out, pools):
    cpool = pools["cpool"]
    wk = pools["wk"]
    spool = pools["spool"]
    gpool = pools["gpool"]
    ps_sc = pools["ps_sc"]
    ps_g = pools["ps_g"]
    ps_h1 = pools["ps_h1"]
    ps_ms = pools["ps_ms"]

    AF = mybir.ActivationFunctionType
    OP = mybir.AluOpType

    # ---- resident constants / staged inputs ----
    def cload(name, shape, dt, src):
        t = cpool.tile(list(shape), dt, name=f"c_{name}")
        nc.sync.dma_start(t[:], src)
        return t

    sb_watt = [cload(f"watt{ch}", (100, 384), BF16, ap["watt"][ch])
               for ch in range(2)]
    H0, HD = SEQ, 8 * SEQ   # xt heads: dialogue 0, then 1-7
    N0, HN = D, 8 * D
    sb_xt = []
    for ch in range(2):
        t = cpool.tile([101, BPC * SEQ], BF16, name=f"c_xt{ch}")
        nc.sync.dma_start(t[:, 0:H0], ap["xtb"][ch][:, 0:H0])
        sb_xt.append(t)
    sb_mk = [cload(f"mk{k}", (128, 2 * BPC), F32, ap["mskb"][k])
             for k in range(3)]
    sb_dir = {(dd, k): cload(f"dir{dd}_{k}", (128, BW), F32, ap["dirb"][dd, k])
              for dd in range(2) for k in range(3)}
    sb_xn = []
    for k in range(3):
        t = cpool.tile([128, BPC * D], BF16, name=f"c_xn{k}")
        nc.sync.dma_start(t[:, 0:N0], ap["xnb"][k][:, 0:N0])
        sb_xn.append(t)
    sb_w8 = cload("w8", (100, 8 * 2 * H), FP8,
                  ap["w8"].transpose([1, 0, 2]))
    sb_root = [cload(f"root{ch}", (101, H), BF16, ap["rootm"][ch])
               for ch in range(2)]
    for ch in range(2):
        nc.sync.dma_start(sb_xt[ch][:, H0:HD], ap["xtb"][ch][:, H0:HD])
    for k in range(3):
        nc.sync.dma_start(sb_xn[k][:, N0:HN], ap["xnb"][k][:, N0:HN])
    sb_win = [cload(f"win{k}", (128, BW), BF16, ap["winb"][k]) for k in range(3)]
    sb_w1 = cload("w1", (H, H), BF16, ap["w1m"][:])
    sb_w2 = cload("w2", (H, H), BF16, ap["w2m"][:])
    sb_wlind = [cload(f"wlind{ch}", (101, H), BF16, ap["wlind"][ch])
                for ch in range(2)]
    sb_wlinh = cload("wlinh", (H, H), BF16, ap["wlinh"][:])
    sb_wfc = cload("wfc", (H, NCLS), BF16, ap["wfc"][:])
    sb_bgc = cload("bgc", (H, 1), F32, ap["bgc"][:])
    sb_blc = cload("blc", (H, 1), F32, ap["blc"][:])
    sb_bfc8 = cload("bfc8", (128, 8 * 3 * NCLS), F32, ap["bfc8"][:])
    for ch in range(2):
        nc.sync.dma_start(sb_xt[ch][:, HD:], ap["xtb"][ch][:, HD:])
    for k in range(3):
        nc.sync.dma_start(sb_xn[k][:, HN:], ap["xnb"][k][:, HN:])
    l_out = cpool.tile([128, BPC * 3 * NCLS], F32, name="c_lout")  # (128, 576)

    # ---- per-dialogue pipeline, software-pipelined 3 stages deep ----
    xpool = pools["xpool"]
    fpool = pools["fpool"]

    def S1(b):
        """scale -> P -> Shat/sums -> xr. Returns cross-stage tiles."""
        psc = ps_sc.tile([128, 340], F32, name="psc", tag="psc")
        for k, (u0, uk) in enumerate(UT):
            L, R = BND[k]
            cof = [0, 138, 286][k]
            for ch in range(2):
                nc.tensor.matmul(
                    psc[:, cof:cof + (R - L)],
                    sb_watt[ch][:, k * 128:(k + 1) * 128],
                    sb_xt[ch][:100, b * SEQ + L:b * SEQ + R],
                    start=(ch == 0), stop=(ch == 1))
        sb_p = wk.tile([128, 340], F32, name="p")
        nc.scalar.activation(sb_p[:], psc[:], AF.Exp)

        sb_s = {}
        acc = wk.tile([128, 6], F32, name="acc")
        for dd in range(2):
            st = spool.tile([128, 340], BF16, name=f"shat{dd}")
            for k, (u0, uk) in enumerate(UT):
                L, R = BND[k]
                cof = [0, 138, 286][k]
                nc.vector.scalar_tensor_tensor(
                    st[:uk, cof:cof + (R - L)], sb_p[:uk, cof:cof + (R - L)],
                    1.0, sb_dir[(dd, k)][:uk, 0:R - L],
                    op0=OP.mult, op1=OP.mult,
                    accum_out=acc[:uk, k * 2 + dd:k * 2 + dd + 1])
            sb_s[dd] = st

        sm = wk.tile([128, 3], F32, name="sm")
        for k, (u0, uk) in enumerate(UT):
            nc.gpsimd.tensor_tensor(sm[:uk, k:k + 1], acc[:uk, k * 2:k * 2 + 1],
                                    acc[:uk, k * 2 + 1:k * 2 + 2],
                                    op=OP.add)
        rc = wk.tile([128, 3], F32, name="rc")
        nc.vector.reciprocal(rc[:, :], sm[:, :])
        sb_xr = {}
        for k, (u0, uk) in enumerate(UT):
            for s in range(2):
                xr = xpool.tile([128, D], BF16, name=f"xr{s}{k}")
                nc.vector.tensor_scalar(
                    xr[:uk, :], sb_xn[k][:uk, b * D:(b + 1) * D],
                    rc[:uk, k:k + 1],
                    sb_mk[k][:uk, s * BPC + b:s * BPC + b + 1],
                    op0=OP.mult, op1=OP.mult)
                sb_xr[(s, k)] = xr
        return sb_s, sb_xr

    def S2(b, sb_s, sb_xr):
        """banded G streams + h1 projections + tau-select. Returns h1f."""
        tmb = wk.tile([128, SEQ], BF16, name="tmb", bufs=3)
        nc.gpsimd.dma_start(
            tmb[:], ap["mskrow"][0, b * SEQ:(b + 1) * SEQ]
            .unsqueeze(0).partition_broadcast(128))
        sb_g = {}
        ph1 = [ps_h1.tile([H, SEQ], F32, name="ph1", tag="ph1")
               for _ in range(2)]
        first = [True, True]

        def emit_h1(s, dd):
            for tau in range(2):
                r = s * 4 + tau * 2 + dd
                nc.tensor.matmul(
                    ph1[tau][:, :],
                    sb_w8[:, r * 2 * H:(r + 1) * 2 * H]
                    .rearrange("p (t f) -> p t f", t=2),
                    sb_g[(s, dd)].rearrange("p (t f) -> p t f", t=2),
                    start=first[tau], stop=False,
                    perf_mode=mybir.MatmulPerfMode.DoubleRow)
                first[tau] = False

        gi = 0
        pend = []
        for s in range(2):
            for dd in range(2):
                g = gpool.tile([100, 2 * SEQ], FP8, name="g", tag="g")
                for ch in range(2):
                    pg = ps_g.tile([128, SEQ], F32, name="pg", tag="pg")
                    for (k, c0, c1, st_, sp_) in GSPLIT:
                        u0, uk = UT[k]
                        L, _ = BND[k]
                        cof = [0, 138, 286][k]
                        nc.tensor.matmul(
                            pg[:100, c0:c1],
                            sb_xr[(s, k)][:uk, ch * 100:(ch + 1) * 100],
                            sb_s[dd][:uk, cof + c0 - L:cof + c1 - L],
                            start=st_, stop=sp_)
                    gh = g[:, ch * SEQ:(ch + 1) * SEQ]
                    if gi in (0, 2, 4, 6):
                        nc.scalar.activation(gh, pg[:100, :], AF.Identity,
                                             scale=16.0)
                    else:
                        nc.vector.tensor_scalar_mul(gh, pg[:100, :], 16.0)
                    gi += 1
                sb_g[(s, dd)] = g
                pend.append((s, dd))
                if len(pend) >= 3:
                    emit_h1(*pend.pop(0))
        while pend:
            emit_h1(*pend.pop(0))
        for tau in range(2):
            for ch in range(2):
                nc.tensor.matmul(ph1[tau][:, :], sb_root[ch][:],
                                 sb_xt[ch][:, b * SEQ:(b + 1) * SEQ],
                                 start=False, stop=(ch == 1))

        sb_h1f = fpool.tile([H, 384], BF16, name="h1f")
        nc.gpsimd.memset(sb_h1f[:, SEQ:384], 0.0)
        nc.scalar.copy(sb_h1f[:, 0:SEQ], ph1[1][:])
        nc.vector.copy_predicated(
            sb_h1f[:, 0:SEQ], tmb.bitcast(mybir.dt.int16)[:],
            ph1[0][:])
        return sb_h1f

    def S3(b, sb_h1f):
        """qT/h2/hidden/logits/transpose into l_out."""
        pqt = ps_ms.tile([128, 3 * H], F32, name="pqt", tag="s3a")
        for k in range(3):
            nc.tensor.matmul(pqt[:, k * H:(k + 1) * H],
                             sb_h1f[:, k * 128:(k + 1) * 128], sb_w2[:],
                             start=True, stop=True)
        sb_qt = wk.tile([128, 3 * H], BF16, name="qt")
        nc.scalar.copy(sb_qt[:], pqt[:])

        ph2 = ps_ms.tile([H, SEQ], F32, name="ph2", tag="s3b")
        nc.tensor.matmul(ph2[:, :], sb_w1[:], sb_h1f[:, 0:SEQ],
                         start=True, stop=False)
        for (k, c0, c1, st_, sp_) in GSPLIT:
            u0, uk = UT[k]
            L, _ = BND[k]
            nc.tensor.matmul(ph2[:, c0:c1], sb_qt[:uk, k * H:(k + 1) * H],
                             sb_win[k][:uk, c0 - L:c1 - L],
                             start=False, stop=sp_)
        sb_h2 = wk.tile([H, SEQ], BF16, name="h2")
        nc.scalar.activation(sb_h2[:], ph2[:], AF.Identity, bias=sb_bgc[:])

        phid = ps_ms.tile([128, SEQ], F32, name="phid", tag="s3a")
        for ch in range(2):
            nc.tensor.matmul(phid[:H, :], sb_wlind[ch][:],
                             sb_xt[ch][:, b * SEQ:(b + 1) * SEQ],
                             start=(ch == 0), stop=False)
        nc.tensor.matmul(phid[:H, :], sb_wlinh[:], sb_h2[:],
                         start=False, stop=True)
        sb_hid = wk.tile([H, 384], BF16, name="hid")
        nc.gpsimd.memset(sb_hid[:, SEQ:384], 0.0)
        nc.scalar.activation(sb_hid[:, 0:SEQ], phid[:H, :], AF.Relu)

        plt = ps_ms.tile([128, 3 * NCLS], F32, name="plt", tag="s3b")
        for k in range(3):
            nc.tensor.matmul(plt[:, k * NCLS:(k + 1) * NCLS],
                             sb_hid[:, k * 128:(k + 1) * 128], sb_wfc[:],
                             start=True, stop=True)
        nc.scalar.activation(l_out[:, b * 18:(b + 1) * 18], plt[:],
                             AF.Identity, scale=1.0 / SCL)

    # ---- stage 2: batched log-softmax over classes, in 8-dialogue chunks ----
    GC = 8 * 3  # groups per chunk (8 dialogues x 3 u-tiles)
    m96 = cpool.tile([128, BPC * 3], F32, name="c_m96")
    esb = cpool.tile([128, BPC * 3 * NCLS], F32, name="c_esb")
    e2sb = cpool.tile([128, BPC * 3 * NCLS], F32, name="c_e2sb")
    s96 = cpool.tile([128, BPC * 3], F32, name="c_s96")
    lnz = cpool.tile([128, BPC * 3], F32, name="c_lnz")
    lsm = cpool.tile([128, BPC * 3], F32, name="c_lsm")
    osb = cpool.tile([128, BPC * 3 * NCLS], F32, name="c_osb")


    state = {}
    for i in range(BPC + 2):
        if i < BPC:
            state[i] = S1(i)
        if 1 <= i <= BPC:
            state[i - 1] = S2(i - 1, *state[i - 1])
        if i >= 2:
            S3(i - 2, state[i - 2])
            del state[i - 2]
    for c in range(4):
        S4(c)

    S4(3)

    # ---- stage 2: batched log-softmax over classes, in 8-dialogue chunks ----
    GC = 8 * 3  # groups per chunk (8 dialogues x 3 u-tiles)
    m96 = cpool.tile([128, BPC * 3], F32, name="c_m96")
    esb = cpool.tile([128, BPC * 3 * NCLS], F32, name="c_esb")
    e2sb = cpool.tile([128, BPC * 3 * NCLS], F32, name="c_e2sb")
    s96 = cpool.tile([128, BPC * 3], F32, name="c_s96")
    lnz = cpool.tile([128, BPC * 3], F32, name="c_lnz")
    lsm = cpool.tile([128, BPC * 3], F32, name="c_lsm")
    osb = cpool.tile([128, BPC * 3 * NCLS], F32, name="c_osb")

    def S4(c):
        GC = 8 * 3
        g0 = c * GC
        l3 = l_out[:, g0 * NCLS:(g0 + GC) * NCLS].rearrange(
            "p (g c) -> p g c", c=NCLS)
        nc.vector.tensor_tensor(l_out[:, g0 * NCLS:(g0 + GC) * NCLS],
                                l_out[:, g0 * NCLS:(g0 + GC) * NCLS],
                                sb_bfc8[:], op=OP.add)
        mc = m96[:, g0:g0 + GC]
        nc.vector.reduce_max(mc, l3, axis=mybir.AxisListType.X)
        e3 = esb[:, g0 * NCLS:(g0 + GC) * NCLS].rearrange(
            "p (g c) -> p g c", c=NCLS)
        for cc in range(NCLS):
            nc.vector.tensor_tensor(e3[:, :, cc], l3[:, :, cc], mc, op=OP.subtract)
        nc.scalar.activation(e2sb[:, g0 * NCLS:(g0 + GC) * NCLS],
                             esb[:, g0 * NCLS:(g0 + GC) * NCLS], AF.Exp)
        nc.vector.reduce_sum(
            s96[:, g0:g0 + GC],
            e2sb[:, g0 * NCLS:(g0 + GC) * NCLS].rearrange(
                "p (g c) -> p g c", c=NCLS),
            axis=mybir.AxisListType.X)
        nc.scalar.activation(lnz[:, g0:g0 + GC], s96[:, g0:g0 + GC], AF.Ln)
        nc.vector.tensor_tensor(lsm[:, g0:g0 + GC], mc, lnz[:, g0:g0 + GC],
                                op=OP.add)
        o3 = osb[:, g0 * NCLS:(g0 + GC) * NCLS].rearrange(
            "p (g c) -> p g c", c=NCLS)
        for cc in range(NCLS):
            nc.vector.tensor_tensor(o3[:, :, cc], l3[:, :, cc],
                                    lsm[:, g0:g0 + GC], op=OP.subtract)
        nc.sync.dma_start(out[:, g0 * NCLS:(g0 + GC) * NCLS],
                          osb[:, g0 * NCLS:(g0 + GC) * NCLS])

    state = {}
    for i in range(BPC + 2):
        if i < BPC:
            state[i] = S1(i)
        if 1 <= i <= BPC:
            state[i - 1] = S2(i - 1, *state[i - 1])
        if i >= 2:
            S3(i - 2, state[i - 2])
            del state[i - 2]
    for c in range(4):
        S4(c)

    S4(3)

    # ---- stage 2: batched log-softmax over classes, in 8-dialogue chunks ----
    GC = 8 * 3  # groups per chunk (8 dialogues x 3 u-tiles)
    m96 = cpool.tile([128, BPC * 3], F32, name="c_m96")
    esb = cpool.tile([128, BPC * 3 * NCLS], F32, name="c_esb")
    e2sb = cpool.tile([128, BPC * 3 * NCLS], F32, name="c_e2sb")
    s96 = cpool.tile([128, BPC * 3], F32, name="c_s96")
    lnz = cpool.tile([128, BPC * 3], F32, name="c_lnz")
    lsm = cpool.tile([128, BPC * 3], F32, name="c_lsm")
    osb = cpool.tile([128, BPC * 3 * NCLS], F32, name="c_osb")

    def S4all():
        GCW = 8 * 3 * NCLS  # 144 cols per chunk
        def sl(t, c):
            return t[:, c * GCW:(c + 1) * GCW]
        def gv(t, c):
            return sl(t, c).rearrange("p (g c) -> p g c", c=NCLS)
        def gs(t, c):
            return t[:, c * GC:(c + 1) * GC]
        for c in range(4):
            nc.vector.tensor_tensor(sl(l_out, c), sl(l_out, c), sb_bfc8[:],
                                    op=OP.add)
        for c in range(4):
            nc.vector.reduce_max(gs(m96, c), gv(l_out, c),
                                 axis=mybir.AxisListType.X)
        for c in range(4):
            for cc in range(NCLS):
                nc.vector.tensor_tensor(gv(esb, c)[:, :, cc],
                                        gv(l_out, c)[:, :, cc], gs(m96, c),
                                        op=OP.subtract)
        for c in range(4):
            nc.scalar.activation(sl(e2sb, c), sl(esb, c), AF.Exp)
        for c in range(4):
            nc.vector.reduce_sum(gs(s96, c), gv(e2sb, c),
                                 axis=mybir.AxisListType.X)
        for c in range(4):
            nc.scalar.activation(gs(lnz, c), gs(s96, c), AF.Ln)
        for c in range(4):
            nc.vector.tensor_tensor(gs(lsm, c), gs(m96, c), gs(lnz, c),
                                    op=OP.add)
        for c in range(4):
            for cc in range(NCLS):
                nc.vector.tensor_tensor(gv(osb, c)[:, :, cc],
                                        gv(l_out, c)[:, :, cc], gs(lsm, c),
                                        op=OP.subtract)
        for c in range(4):
            nc.sync.dma_start(sl(out, c), sl(osb, c))

    state = {}
    for i in range(BPC + 2):
        if i < BPC:
            state[i] = S1(i)
        if 1 <= i <= BPC:
            state[i - 1] = S2(i - 1, *state[i - 1])
        if i >= 2:
            S3(i - 2, state[i - 2])
            del state[i - 2]
    for c in range(4):
        S4(c)

    S4(3)

    # ---- stage 2: batched log-softmax over classes, in 8-dialogue chunks ----
    GC = 8 * 3  # groups per chunk (8 dialogues x 3 u-tiles)
    m96 = cpool.tile([128, BPC * 3], F32, name="c_m96")
    esb = cpool.tile([128, BPC * 3 * NCLS], F32, name="c_esb")
    e2sb = cpool.tile([128, BPC * 3 * NCLS], F32, name="c_e2sb")
    s96 = cpool.tile([128, BPC * 3], F32, name="c_s96")
    lnz = cpool.tile([128, BPC * 3], F32, name="c_lnz")
    lsm = cpool.tile([128, BPC * 3], F32, name="c_lsm")
    osb = cpool.tile([128, BPC * 3 * NCLS], F32, name="c_osb")

    def S4(c):
        g0 = c * GC
        l3 = l_out[:, g0 * NCLS:(g0 + GC) * NCLS].rearrange(
            "p (g c) -> p g c", c=NCLS)
        nc.vector.tensor_tensor(l_out[:, g0 * NCLS:(g0 + GC) * NCLS],
                                l_out[:, g0 * NCLS:(g0 + GC) * NCLS],
                                sb_bfc8[:], op=OP.add)
        mc = m96[:, g0:g0 + GC]
        nc.vector.reduce_max(mc, l3, axis=mybir.AxisListType.X)
        e3 = esb[:, g0 * NCLS:(g0 + GC) * NCLS].rearrange(
            "p (g c) -> p g c", c=NCLS)
        for cc in range(NCLS):
            nc.vector.tensor_tensor(e3[:, :, cc], l3[:, :, cc], mc, op=OP.subtract)
        nc.scalar.activation(e2sb[:, g0 * NCLS:(g0 + GC) * NCLS],
                             esb[:, g0 * NCLS:(g0 + GC) * NCLS], AF.Exp)
        nc.vector.reduce_sum(
            s96[:, g0:g0 + GC],
            e2sb[:, g0 * NCLS:(g0 + GC) * NCLS].rearrange(
                "p (g c) -> p g c", c=NCLS),
            axis=mybir.AxisListType.X)
        nc.scalar.activation(lnz[:, g0:g0 + GC], s96[:, g0:g0 + GC], AF.Ln)
        nc.vector.tensor_tensor(lsm[:, g0:g0 + GC], mc, lnz[:, g0:g0 + GC],
                                op=OP.add)
        o3 = osb[:, g0 * NCLS:(g0 + GC) * NCLS].rearrange(
            "p (g c) -> p g c", c=NCLS)
        for cc in range(NCLS):
            nc.vector.tensor_tensor(o3[:, :, cc], l3[:, :, cc],
                                    lsm[:, g0:g0 + GC], op=OP.subtract)
        nc.sync.dma_start(out[:, g0 * NCLS:(g0 + GC) * NCLS],
                          osb[:, g0 * NCLS:(g0 + GC) * NCLS])



def _host_prep(inputs):
    feats = np.asarray(inputs["features"], dtype=np.float32)    # (300,256,200)
    spk = np.asarray(inputs["speakers"])                        # (300,256)
    W_att = np.asarray(inputs["W_att"], dtype=np.float32)
    basis = np.asarray(inputs["basis"], dtype=np.float32)
    comp = np.asarray(inputs["comp"], dtype=np.float32)
    root = np.asarray(inputs["root"], dtype=np.float32)
    bias_r = np.asarray(inputs["bias_r"], dtype=np.float32)
    W1 = np.asarray(inputs["W1"], dtype=np.float32)
    W2 = np.asarray(inputs["W2"], dtype=np.float32)
    b_gc = np.asarray(inputs["b_gc"], dtype=np.float32)
    W_lin = np.asarray(inputs["W_lin"], dtype=np.float32)
    b_lin = np.asarray(inputs["b_lin"], dtype=np.float32)
    W_fc = np.asarray(inputs["W_fc"], dtype=np.float32)
    b_fc = np.asarray(inputs["b_fc"], dtype=np.float32)

    def bf(a):
        return np.ascontiguousarray(a).astype(NPBF16)

    i = np.arange(SEQ)[:, None]
    j = np.arange(SEQ)[None, :]
    win = (j >= i - WP) & (j <= i + WF)
    dir0 = (win & (i < j)).astype(np.float32)
    dir1 = (win & (i >= j)).astype(np.float32)
    winm = win.astype(np.float32)

    dirb = np.zeros((2, 3, 128, BW), np.float32)
    winb = np.zeros((3, 128, BW), np.float32)
    for k, (u0, uk) in enumerate(UT):
        L, R = BND[k]
        dirb[0, k, :uk, :R - L] = dir0[u0:u0 + uk, L:R]
        dirb[1, k, :uk, :R - L] = dir1[u0:u0 + uk, L:R]
        winb[k, :uk, :R - L] = winm[u0:u0 + uk, L:R]

    w = np.einsum("rb,bdh->rdh", comp, basis).astype(np.float32)  # (8,200,128)
    w8 = (w * (SCL / 16.0)).reshape(8, 2, 100, H).transpose(0, 2, 1, 3) \
        .reshape(8, 100, 2 * H).astype(NPFP8)

    shared = {
        "dirb": dirb, "winb": bf(winb),
        "watt": bf(np.concatenate(
            [W_att.reshape(2, 100, SEQ),
             np.zeros((2, 100, 384 - SEQ), np.float32)], axis=2)),
        "w8": w8,
        "rootm": bf(SCL * np.concatenate(
            [root.reshape(2, 100, H),
             np.stack([np.zeros((1, H), np.float32),
                       bias_r.reshape(1, H)])], axis=1)),
        "w1m": bf(W1), "w2m": bf(W2),
        "wlind": bf(SCL * np.concatenate(
            [W_lin[:D].reshape(2, 100, H),
             np.stack([np.zeros((1, H), np.float32),
                       b_lin.reshape(1, H)])], axis=1)),
        "wlinh": bf(W_lin[D:]), "wfc": bf(W_fc),
        "bgc": SCL * b_gc.reshape(H, 1),
        "blc": b_lin.reshape(H, 1),
        "bfc8": np.broadcast_to(np.tile(b_fc, 8 * 3), (128, 144)).copy(),
    }

    in_maps = []
    for c in range(NCORES):
        bs = slice(c * BPC, (c + 1) * BPC)
        fb = feats[:, bs, :]                                    # (300,32,200)
        sp = spk[:, bs]                                         # (300,32)
        xtb = np.zeros((2, 101, BPC * SEQ), NPBF16)
        xtb[:, :100] = bf(fb.transpose(2, 1, 0).reshape(2, 100, BPC * SEQ))
        xtb[1, 100] = NPBF16(1.0)
        xnb = np.zeros((3, 128, BPC * D), NPBF16)
        mskb = np.zeros((3, 128, 2 * BPC), np.float32)
        for k, (u0, uk) in enumerate(UT):
            xnb[k, :uk] = bf(fb[u0:u0 + uk].reshape(uk, BPC * D))
            mm = np.stack([(sp[u0:u0 + uk] == 0), (sp[u0:u0 + uk] == 1)], 1)
            mskb[k, :uk] = mm.astype(np.float32).reshape(uk, 2 * BPC)
        mskrow = np.stack([(sp.T == 0), (sp.T == 1)]).astype(NPBF16).reshape(2, BPC * SEQ)
        m = {"xtb": xtb, "xnb": xnb, "mskb": mskb, "mskrow": mskrow}
        m.update(shared)
        in_maps.append(m)
    return in_maps


def get_program():
    if "nc" not in _CACHE:
        _CACHE["nc"] = _build_program()
    return _CACHE["nc"]


def kernel(**inputs):
    nc = get_program()
    in_maps = _host_prep(inputs)
    res = bass_utils.run_bass_kernel_spmd(nc, in_maps, core_ids=list(range(NCORES)))
    full = np.empty((NCORES * BPC * SEQ, NCLS), np.float32)
    for c in range(NCORES):
        osb = res.results[c]["out"]                     # (128, 576)
        o4 = osb.reshape(128, BPC, 3, NCLS)
        base = c * BPC * SEQ
        for k, (u0, uk) in enumerate(UT):
            for b in range(BPC):
                full[base + b * SEQ + u0:base + b * SEQ + u0 + uk, :] = \
                    o4[:uk, b, k, :]
    return full
